# revision 33
# baseline (speedup 1.0000x reference)
"""GPNConv (GNN message passing) Trainium2 Bass kernel.

  agg = segment_sum(x[col], row, N)        # [N, 128]
  out = (x + agg) @ W.T + b                # [N, 512]

Sharding: destination nodes split across 8 cores (12500 each); no
cross-core communication. Per core, edges are grouped by 256-node
destination window ("pair") and by 25000-row source bucket (dma_gather
indices are int16). Each core bulk-gathers its neighbor rows x[col]
from a replicated x via dma_gather, segment-sums them with one-hot
matmuls on the PE (one-hot built on-chip from dest slots via is_equal
against a host-supplied iota row), adds the residual from a
host-transposed x-shard, applies the 128->512 linear + bias, and
writes its output shard in bf16.

Destinations are bin-packed on the host: each core's 12500 dest nodes
are assigned to its 49 pairs so that every (pair, bucket) edge count
stays at/below a multiple of 128, which keeps the uniform cross-core
chunk schedule near the 75000/128 minimum. The dest permutation is
undone on the host after the run. Gather indices are sorted within
each group for HBM locality.
"""

import hashlib
import os

import numpy as np

import concourse.bass as bass
import concourse.mybir as mybir
import concourse.tile as tile
from concourse import bacc
from concourse import bass_utils

P = 128
N_NODES = 100000
D_IN = 128
D_OUT = 512
N_CORES = 8
NODES_PER_CORE = N_NODES // N_CORES             # 12500
DPAIR = 256                                      # dest window (psum free dim)
PAIRS_PER_CORE = 50                              # 12800 slots for 12500 dests
PAD_NODES = PAIRS_PER_CORE * DPAIR               # 12800
WAVE_SIZES = [4, 8, 12, 12, 10, 4]               # pairs per gather wave
N_BUCKETS = 4
SRC_BUCKET = 25000                               # rows per source bucket (int16 range)
CAP = 384                                        # max edges per (pair, bucket): 3 chunks
PAD_SLOT = 999.0                                 # one-hot slot matching nothing
SCHED_CACHE = "/tmp/gpn_sched_v3.npz"

_F32 = mybir.dt.float32
_BF16 = mybir.dt.bfloat16
_I16 = mybir.dt.int16


def _assign_dests(row, col):
    """Assign dest nodes to (core, pair, slot) so that per-(core,pair,bucket)
    edge counts pack tightly into 128-edge chunks.

    Returns perm[core, PAD_NODES] = global dest id per slot (-1 for empty)
    and counts[core, PAIRS, NB]."""
    bucket = col // SRC_BUCKET
    # per-dest degree vector over buckets
    dv = np.zeros((N_NODES, N_BUCKETS), dtype=np.int32)
    np.add.at(dv, (row, bucket), 1)
    deg = dv.sum(axis=1)

    # dest -> core: snake-deal in degree-desc order balances per-core bucket sums
    order = np.argsort(-deg, kind="stable")
    core_of = np.empty(N_NODES, dtype=np.int32)
    pos = np.arange(N_NODES)
    rnd = pos // N_CORES
    lane = pos % N_CORES
    lane = np.where(rnd % 2 == 1, N_CORES - 1 - lane, lane)
    core_of[order] = lane

    perm = np.full((N_CORES, PAD_NODES), -1, dtype=np.int64)
    counts = np.zeros((N_CORES, PAIRS_PER_CORE, N_BUCKETS), dtype=np.int32)
    for c in range(N_CORES):
        dests = np.where(core_of == c)[0]
        dvc = dv[dests]
        dorder = np.argsort(-dvc.sum(axis=1), kind="stable")
        cnt = np.zeros((PAIRS_PER_CORE, N_BUCKETS), dtype=np.int64)
        caps = np.full((PAIRS_PER_CORE, N_BUCKETS), CAP, dtype=np.int64)
        for b in range(N_BUCKETS):
            caps[2 * b : 2 * b + 2, b] = 256
        slots = np.zeros(PAIRS_PER_CORE, dtype=np.int64)
        assign = np.empty(len(dests), dtype=np.int64)
        for i in dorder:
            v = dvc[i]
            new = cnt + v[None, :]
            # balance buckets within the hard cap; prefer the fullest
            # feasible pair so later (big) dests keep room
            score = (new / caps).max(axis=1)
            score[(new > caps).any(axis=1)] = 1e18
            score[slots >= DPAIR] = 1e18
            p = int(np.argmin(score))
            if score[p] >= 1e18:  # no feasible pair: least-overflow fallback
                over = np.maximum(new - caps, 0).sum(axis=1).astype(np.float64)
                over[slots >= DPAIR] = 1e18
                p = int(np.argmin(over))
            assign[i] = p
            cnt[p] += v
            slots[p] += 1
        # slot positions within each pair, in assignment order
        for p in range(PAIRS_PER_CORE):
            mask = assign == p
            ds = dests[mask]
            perm[c, p * DPAIR : p * DPAIR + len(ds)] = ds
        counts[c] = cnt
    return perm, counts


def _host_prep(edge_index):
    """Group edges by (core, pair, bucket) under a bin-packed dest layout;
    build uniform chunk schedule, int16 gather-index array (sorted within
    groups) and bf16 dest-slot array per core."""
    row = np.asarray(edge_index[0], dtype=np.int64)
    col = np.asarray(edge_index[1], dtype=np.int64)

    key_h = hashlib.sha1(np.ascontiguousarray(edge_index)).hexdigest()
    perm = None
    if os.path.exists(SCHED_CACHE):
        try:
            z = np.load(SCHED_CACHE, allow_pickle=False)
            if str(z["key"]) == key_h:
                perm, counts = z["perm"], z["counts"]
        except Exception:
            perm = None
    if perm is None:
        perm, counts = _assign_dests(row, col)
        try:
            np.savez(SCHED_CACHE, key=key_h, perm=perm, counts=counts)
        except Exception:
            pass

    # slot_of[global dest] = core * PAD_NODES + slot
    slot_of = np.empty(N_NODES, dtype=np.int64)
    valid = perm >= 0
    flat_idx = np.arange(N_CORES * PAD_NODES).reshape(N_CORES, PAD_NODES)
    slot_of[perm[valid]] = flat_idx[valid]

    s = slot_of[row]
    core = s // PAD_NODES
    local = s % PAD_NODES
    pair = local // DPAIR                         # 0..48
    pslot = local % DPAIR                         # 0..255
    bucket = col // SRC_BUCKET
    brel = (col % SRC_BUCKET).astype(np.int16)

    key = (core * PAIRS_PER_CORE + pair) * N_BUCKETS + bucket
    ngroups = N_CORES * PAIRS_PER_CORE * N_BUCKETS
    budget = -(-counts.max(axis=0) // P)          # [PAIRS, NB] ceil

    # sort edges by (group, brel) for HBM locality within each chunk run
    order = np.lexsort((brel, key))
    brel_s = brel[order]
    pslot_s = pslot[order]
    key_s = key[order]
    starts = np.searchsorted(key_s, np.arange(ngroups + 1))

    assert sum(WAVE_SIZES) == PAIRS_PER_CORE
    waves = []
    p0 = 0
    for ws in WAVE_SIZES:
        waves.append(list(range(p0, p0 + ws)))
        p0 += ws
    gathers = []      # per (w,b): dict(nch, qoff, coff, bucket, wave)
    pair_chunks = {}  # pair -> list of (gather idx, local_chunk, ci)
    ci = 0
    qcols = 0
    for w, wp in enumerate(waves):
        for b in range(N_BUCKETS):
            nch = int(sum(budget[p][b] for p in wp))
            if nch == 0:
                continue
            g = dict(w=w, b=b, nch=nch, qoff=qcols, coff=ci)
            gathers.append(g)
            lc = 0
            for p in wp:
                for j in range(int(budget[p][b])):
                    pair_chunks.setdefault(p, []).append((len(gathers) - 1, lc, ci))
                    lc += 1
                    ci += 1
            qcols += nch * 8  # (nch*128 idxs)/16
    TC = ci

    idx_all = np.zeros((N_CORES, 16, qcols), dtype=np.int16)
    dfl_all = np.full((N_CORES, P, TC), PAD_SLOT, dtype=np.float32)
    for c in range(N_CORES):
        for g in gathers:
            w, b = g["w"], g["b"]
            lc = 0
            for p in waves[w]:
                gk = (c * PAIRS_PER_CORE + p) * N_BUCKETS + b
                b0, b1 = starts[gk], starts[gk + 1]
                n = b1 - b0
                kb = int(budget[p][b])
                assert n <= kb * P, (c, p, b, n, kb)
                if n:
                    i = (lc + np.arange(n) // P) * P + np.arange(n) % P
                    idx_all[c, i % 16, g["qoff"] + i // 16] = brel_s[b0:b1]
                    dfl_all[c, np.arange(n) % P, g["coff"] + lc + np.arange(n) // P] = (
                        pslot_s[b0:b1]
                    )
                lc += kb
    # replicate idx rows to 128 partitions (8 Q7 cores x 16-partition stripes)
    idx_all = np.tile(idx_all, (1, 8, 1))
    return idx_all, dfl_all, gathers, pair_chunks, waves, TC, qcols, perm


def _build_program(gathers, pair_chunks, waves, TC, qcols):
    nc = bacc.Bacc(
        "TRN2",
        target_bir_lowering=False,
        debug=False,
        enable_asserts=False,
        num_devices=N_CORES,
        num_swdge_queues=4,
        dynamic_dma_scratch_size=24576,
    )
    x_d = nc.dram_tensor("x", [N_NODES, D_IN], _BF16, kind="ExternalInput").ap()
    idx_d = nc.dram_tensor("idx", [P, qcols], _I16, kind="ExternalInput").ap()
    dfl_d = nc.dram_tensor("dfl", [P, TC], _F32, kind="ExternalInput").ap()
    xt_d = nc.dram_tensor("xt", [P, PAD_NODES], _F32, kind="ExternalInput").ap()
    wt_d = nc.dram_tensor("wt", [P, D_OUT], _BF16, kind="ExternalInput").ap()
    iota_d = nc.dram_tensor("iota", [P, DPAIR], _BF16, kind="ExternalInput").ap()
    bcol_d = nc.dram_tensor("bcol", [P, D_OUT // P], _F32, kind="ExternalInput").ap()
    out_d = nc.dram_tensor("out", [D_OUT, PAD_NODES], _BF16, kind="ExternalOutput").ap()

    with tile.TileContext(nc) as tc:
        with (
            tc.tile_pool(name="const", bufs=1) as cpool,
            tc.tile_pool(name="gather", bufs=3) as gpool,
            tc.tile_pool(name="xtw", bufs=2) as xtpool,
            tc.tile_pool(name="oh", bufs=12) as ohpool,
            tc.tile_pool(name="ht", bufs=6) as htpool,
            tc.tile_pool(name="ot", bufs=3) as otpool,
            tc.tile_pool(name="psA", bufs=6, space="PSUM") as psA,
            tc.tile_pool(name="psB", bufs=2, space="PSUM") as psB,
        ):
            wt_t = cpool.tile([P, D_OUT], _BF16)
            nc.sync.dma_start(out=wt_t[:], in_=wt_d)
            bcol_t = cpool.tile([P, D_OUT // P], _F32)
            nc.sync.dma_start(out=bcol_t[:], in_=bcol_d)
            iota_t = cpool.tile([P, DPAIR], _BF16)
            nc.sync.dma_start(out=iota_t[:], in_=iota_d)
            warm_idx = cpool.tile([P, 8], _I16)
            nc.vector.memset(warm_idx[:], 0)
            warm_gt = cpool.tile([P, P], _BF16)
            nc.gpsimd.dma_gather(
                warm_gt[:].rearrange("p (c e) -> p c e", e=P),
                x_d[0:SRC_BUCKET, :],
                warm_idx[:],
                P,
                P,
                P,
                single_packet=False,
                queue_num=0,
            )
            wave_q = {}
            for g in gathers:
                w = g["w"]
                q0, q1 = wave_q.get(w, (g["qoff"], g["qoff"]))
                wave_q[w] = (min(q0, g["qoff"]), max(q1, g["qoff"] + g["nch"] * 8))
            idx_ts = {}
            for w, (q0, q1) in sorted(wave_q.items()):
                t = cpool.tile([P, q1 - q0], _I16, tag=f"idx{w}")
                nc.sync.dma_start(out=t[:], in_=idx_d[:, q0:q1])
                idx_ts[w] = (t, q0)
            dfl_t = cpool.tile([P, TC], _F32)
            nc.sync.dma_start(out=dfl_t[:], in_=dfl_d)
            ndfl_t = cpool.tile([P, TC], _F32)
            nc.vector.tensor_scalar(
                out=ndfl_t[:], in0=dfl_t[:], scalar1=-1.0, scalar2=None,
                op0=mybir.AluOpType.mult,
            )

            for w, wp in enumerate(waves):
                gts = {}
                wgs = [g for g in gathers if g["w"] == w]
                for g in sorted(wgs, key=lambda g: (g["b"] == 0)):
                    b = g["b"]
                    nch = g["nch"]
                    b0 = b * SRC_BUCKET
                    b1 = min(b0 + SRC_BUCKET, N_NODES)
                    gt = gpool.tile([P, nch * P], _BF16, tag=f"g{b}")
                    nc.gpsimd.dma_gather(
                        gt[:].rearrange("p (c e) -> p c e", e=P),
                        x_d[b0:b1, :],
                        idx_ts[w][0][:, g["qoff"] - idx_ts[w][1] : g["qoff"] - idx_ts[w][1] + nch * 8],
                        nch * P,
                        nch * P,
                        P,
                        single_packet=False,
                        queue_num=b,
                    )
                    gts[b] = gt
                xtw = xtpool.tile([P, len(wp) * DPAIR], _F32)
                nc.sync.dma_start(
                    out=xtw[:],
                    in_=xt_d[:, wp[0] * DPAIR : (wp[-1] + 1) * DPAIR],
                )
                assert len(wp) % 2 == 0
                for pi, p in enumerate(wp):
                    if pi % 2 == 0:
                        ht = htpool.tile([P, 2 * DPAIR], _BF16)
                    chunks = pair_chunks[p]
                    psT = psA.tile([P, DPAIR], _F32)
                    for k, (gi, lc, ci) in enumerate(chunks):
                        oh = ohpool.tile([P, DPAIR], _BF16)
                        if k % 5 == 4:
                            tmp = ohpool.tile([P, DPAIR], _BF16, tag="ohtmp")
                            nc.scalar.activation(
                                out=tmp[:],
                                in_=iota_t[:],
                                func=mybir.ActivationFunctionType.Abs,
                                bias=ndfl_t[:, ci : ci + 1],
                                scale=1.0,
                            )
                            nc.scalar.activation(
                                out=oh[:],
                                in_=tmp[:],
                                func=mybir.ActivationFunctionType.Relu,
                                bias=1.0,
                                scale=-1.0,
                            )
                        else:
                            nc.vector.tensor_scalar(
                                out=oh[:],
                                in0=iota_t[:],
                                scalar1=dfl_t[:, ci : ci + 1],
                                scalar2=None,
                                op0=mybir.AluOpType.is_equal,
                            )
                        nc.tensor.matmul(
                            out=psT[:],
                            lhsT=gts[gathers[gi]["b"]][:, lc * P : (lc + 1) * P],
                            rhs=oh[:],
                            start=(k == 0),
                            stop=(k == len(chunks) - 1),
                        )
                    nc.vector.tensor_tensor(
                        out=ht[:, (pi % 2) * DPAIR : (pi % 2 + 1) * DPAIR],
                        in0=psT[:],
                        in1=xtw[:, pi * DPAIR : (pi + 1) * DPAIR],
                        op=mybir.AluOpType.add,
                    )
                    if pi % 2 == 0:
                        continue
                    ot = otpool.tile([P, (D_OUT // P) * 2 * DPAIR], _BF16)
                    for h in range(D_OUT // P):
                        psO = psB.tile([P, 2 * DPAIR], _F32)
                        nc.tensor.matmul(
                            out=psO[:],
                            lhsT=wt_t[:, h * P : (h + 1) * P],
                            rhs=ht[:],
                            start=True,
                            stop=True,
                        )
                        nc.scalar.activation(
                            out=ot[:, h * 2 * DPAIR : (h + 1) * 2 * DPAIR],
                            in_=psO[:],
                            func=mybir.ActivationFunctionType.Identity,
                            bias=bcol_t[:, h : h + 1],
                            scale=1.0,
                        )
                    nc.sync.dma_start(
                        out=out_d[:, (p - 1) * DPAIR : (p + 1) * DPAIR].rearrange(
                            "(h q) n -> q h n", q=P
                        ),
                        in_=ot[:].rearrange("q (h n) -> q h n", h=D_OUT // P),
                    )
    nc.compile()
    return nc


def _run(inputs, trace=False):
    import ml_dtypes
    x = np.ascontiguousarray(np.asarray(inputs["x"], dtype=np.float32))
    xb = np.ascontiguousarray(x.astype(ml_dtypes.bfloat16))
    W = np.asarray(inputs["W"], dtype=np.float32)
    b = np.asarray(inputs["b"], dtype=np.float32)

    idx_all, dfl_all, gathers, pair_chunks, waves, TC, qcols, perm = _host_prep(
        inputs["edge_index"]
    )

    WT = np.ascontiguousarray(W.T).astype(ml_dtypes.bfloat16)
    bcol = np.ascontiguousarray(b.reshape(D_OUT // P, P).T).astype(np.float32)
    iota = np.ascontiguousarray(
        np.broadcast_to(np.arange(DPAIR, dtype=np.float32)[None, :], (P, DPAIR))
    ).astype(ml_dtypes.bfloat16)

    in_maps = []
    for c in range(N_CORES):
        xt = np.zeros((P, PAD_NODES), dtype=np.float32)
        pc = perm[c]
        v = pc >= 0
        xt[:, v] = x[pc[v]].T
        in_maps.append(
            {
                "x": xb,
                "idx": np.ascontiguousarray(idx_all[c]),
                "dfl": np.ascontiguousarray(dfl_all[c]),
                "xt": xt,
                "wt": WT,
                "bcol": bcol,
                "iota": iota,
            }
        )

    nc = _build_program(gathers, pair_chunks, waves, TC, qcols)
    res = bass_utils.run_bass_kernel_spmd(
        nc, in_maps, core_ids=list(range(N_CORES)), trace=trace
    )
    out = np.empty((N_NODES, D_OUT), dtype=np.float32)
    for c in range(N_CORES):
        pc = perm[c]
        v = pc >= 0
        out[pc[v]] = res.results[c]["out"][:, v].T.astype(np.float32)
    return out, res


def kernel(**inputs):
    out, _ = _run(inputs, trace=False)
    return out


# revision 34
# speedup vs baseline: 1.1682x; 1.1682x over previous
"""GPNConv (GNN message passing) Trainium2 Bass kernel.

  agg = segment_sum(x[col], row, N)        # [N, 128]
  out = (x + agg) @ W.T + b                # [N, 512]

Sharding: destination nodes split across 8 cores (12500 each); no
cross-core communication. Per core, edges are grouped by 256-node
destination window ("pair") and by 25000-row source bucket (dma_gather
indices are int16). Each core bulk-gathers its neighbor rows x[col]
from a replicated x via dma_gather, segment-sums them with one-hot
matmuls on the PE (one-hot built on-chip from dest slots via is_equal
against a host-supplied iota row), adds the residual from a
host-transposed x-shard, applies the 128->512 linear + bias, and
writes its output shard in bf16.

Destinations are bin-packed on the host: each core's 12500 dest nodes
are assigned to its 49 pairs so that every (pair, bucket) edge count
stays at/below a multiple of 128, which keeps the uniform cross-core
chunk schedule near the 75000/128 minimum. The dest permutation is
undone on the host after the run. Gather indices are sorted within
each group for HBM locality.
"""

import hashlib
import os

import numpy as np

import concourse.bass as bass
import concourse.mybir as mybir
import concourse.tile as tile
from concourse import bacc
from concourse import bass_utils

P = 128
N_NODES = 100000
D_IN = 128
D_OUT = 512
N_CORES = 8
NODES_PER_CORE = N_NODES // N_CORES             # 12500
DPAIR = 256                                      # dest window (psum free dim)
PAIRS_PER_CORE = 50                              # 12800 slots for 12500 dests
PAD_NODES = PAIRS_PER_CORE * DPAIR               # 12800
WAVE_SIZES = [4, 8, 12, 12, 10, 4]               # pairs per gather wave
N_BUCKETS = 4
SRC_BUCKET = 25000                               # rows per source bucket (int16 range)
CAP = 384                                        # max edges per (pair, bucket): 3 chunks
PAD_SLOT = 999.0                                 # one-hot slot matching nothing
SCHED_CACHE = "/tmp/gpn_sched_v3.npz"

_F32 = mybir.dt.float32
_BF16 = mybir.dt.bfloat16
_I16 = mybir.dt.int16


def _assign_dests(row, col):
    """Assign dest nodes to (core, pair, slot) so that per-(core,pair,bucket)
    edge counts pack tightly into 128-edge chunks.

    Returns perm[core, PAD_NODES] = global dest id per slot (-1 for empty)
    and counts[core, PAIRS, NB]."""
    bucket = col // SRC_BUCKET
    # per-dest degree vector over buckets
    dv = np.zeros((N_NODES, N_BUCKETS), dtype=np.int32)
    np.add.at(dv, (row, bucket), 1)
    deg = dv.sum(axis=1)

    # dest -> core: snake-deal in degree-desc order balances per-core bucket sums
    order = np.argsort(-deg, kind="stable")
    core_of = np.empty(N_NODES, dtype=np.int32)
    pos = np.arange(N_NODES)
    rnd = pos // N_CORES
    lane = pos % N_CORES
    lane = np.where(rnd % 2 == 1, N_CORES - 1 - lane, lane)
    core_of[order] = lane

    perm = np.full((N_CORES, PAD_NODES), -1, dtype=np.int64)
    counts = np.zeros((N_CORES, PAIRS_PER_CORE, N_BUCKETS), dtype=np.int32)
    for c in range(N_CORES):
        dests = np.where(core_of == c)[0]
        dvc = dv[dests]
        dorder = np.argsort(-dvc.sum(axis=1), kind="stable")
        cnt = np.zeros((PAIRS_PER_CORE, N_BUCKETS), dtype=np.int64)
        slots = np.zeros(PAIRS_PER_CORE, dtype=np.int64)
        assign = np.empty(len(dests), dtype=np.int64)
        for i in dorder:
            v = dvc[i]
            new = cnt + v[None, :]
            # balance buckets within the hard CAP; prefer the fullest
            # feasible pair so later (big) dests keep room
            score = (new / CAP).max(axis=1)
            score[(new > CAP).any(axis=1)] = 1e18
            score[slots >= DPAIR] = 1e18
            p = int(np.argmin(score))
            if score[p] >= 1e18:  # no feasible pair: least-overflow fallback
                over = np.maximum(new - CAP, 0).sum(axis=1).astype(np.float64)
                over[slots >= DPAIR] = 1e18
                p = int(np.argmin(over))
            assign[i] = p
            cnt[p] += v
            slots[p] += 1
        # slot positions within each pair, in assignment order
        for p in range(PAIRS_PER_CORE):
            mask = assign == p
            ds = dests[mask]
            perm[c, p * DPAIR : p * DPAIR + len(ds)] = ds
        counts[c] = cnt
    return perm, counts


def _host_prep(edge_index):
    """Group edges by (core, pair, bucket) under a bin-packed dest layout;
    build uniform chunk schedule, int16 gather-index array (sorted within
    groups) and bf16 dest-slot array per core."""
    row = np.asarray(edge_index[0], dtype=np.int64)
    col = np.asarray(edge_index[1], dtype=np.int64)

    key_h = hashlib.sha1(np.ascontiguousarray(edge_index)).hexdigest()
    perm = None
    if os.path.exists(SCHED_CACHE):
        try:
            z = np.load(SCHED_CACHE, allow_pickle=False)
            if str(z["key"]) == key_h:
                perm, counts = z["perm"], z["counts"]
        except Exception:
            perm = None
    if perm is None:
        perm, counts = _assign_dests(row, col)
        try:
            np.savez(SCHED_CACHE, key=key_h, perm=perm, counts=counts)
        except Exception:
            pass

    # slot_of[global dest] = core * PAD_NODES + slot
    slot_of = np.empty(N_NODES, dtype=np.int64)
    valid = perm >= 0
    flat_idx = np.arange(N_CORES * PAD_NODES).reshape(N_CORES, PAD_NODES)
    slot_of[perm[valid]] = flat_idx[valid]

    s = slot_of[row]
    core = s // PAD_NODES
    local = s % PAD_NODES
    pair = local // DPAIR                         # 0..48
    pslot = local % DPAIR                         # 0..255
    bucket = col // SRC_BUCKET
    brel = (col % SRC_BUCKET).astype(np.int16)

    key = (core * PAIRS_PER_CORE + pair) * N_BUCKETS + bucket
    ngroups = N_CORES * PAIRS_PER_CORE * N_BUCKETS
    budget = -(-counts.max(axis=0) // P)          # [PAIRS, NB] ceil

    # sort edges by (group, brel) for HBM locality within each chunk run
    order = np.lexsort((brel, key))
    brel_s = brel[order]
    pslot_s = pslot[order]
    key_s = key[order]
    starts = np.searchsorted(key_s, np.arange(ngroups + 1))

    assert sum(WAVE_SIZES) == PAIRS_PER_CORE
    waves = []
    p0 = 0
    for ws in WAVE_SIZES:
        waves.append(list(range(p0, p0 + ws)))
        p0 += ws
    gathers = []      # per (w,b): dict(nch, qoff, coff, bucket, wave)
    pair_chunks = {}  # pair -> list of (gather idx, local_chunk, ci)
    ci = 0
    qcols = 0
    for w, wp in enumerate(waves):
        for b in range(N_BUCKETS):
            nch = int(sum(budget[p][b] for p in wp))
            if nch == 0:
                continue
            g = dict(w=w, b=b, nch=nch, qoff=qcols, coff=ci)
            gathers.append(g)
            lc = 0
            for p in wp:
                for j in range(int(budget[p][b])):
                    pair_chunks.setdefault(p, []).append((len(gathers) - 1, lc, ci))
                    lc += 1
                    ci += 1
            qcols += nch * 8  # (nch*128 idxs)/16
    TC = ci

    idx_all = np.zeros((N_CORES, 16, qcols), dtype=np.int16)
    dfl_all = np.full((N_CORES, P, TC), PAD_SLOT, dtype=np.float32)
    for c in range(N_CORES):
        for g in gathers:
            w, b = g["w"], g["b"]
            lc = 0
            for p in waves[w]:
                gk = (c * PAIRS_PER_CORE + p) * N_BUCKETS + b
                b0, b1 = starts[gk], starts[gk + 1]
                n = b1 - b0
                kb = int(budget[p][b])
                assert n <= kb * P, (c, p, b, n, kb)
                if n:
                    i = (lc + np.arange(n) // P) * P + np.arange(n) % P
                    idx_all[c, i % 16, g["qoff"] + i // 16] = brel_s[b0:b1]
                    dfl_all[c, np.arange(n) % P, g["coff"] + lc + np.arange(n) // P] = (
                        pslot_s[b0:b1]
                    )
                lc += kb
    # replicate idx rows to 128 partitions (8 Q7 cores x 16-partition stripes)
    idx_all = np.tile(idx_all, (1, 8, 1))
    return idx_all, dfl_all, gathers, pair_chunks, waves, TC, qcols, perm


def _build_program(gathers, pair_chunks, waves, TC, qcols):
    nc = bacc.Bacc(
        "TRN2",
        target_bir_lowering=False,
        debug=False,
        enable_asserts=False,
        num_devices=N_CORES,
        num_swdge_queues=4,
        dynamic_dma_scratch_size=32768,
    )
    x_d = nc.dram_tensor("x", [N_NODES, D_IN], _BF16, kind="ExternalInput").ap()
    idx_d = nc.dram_tensor("idx", [P, qcols], _I16, kind="ExternalInput").ap()
    dfl_d = nc.dram_tensor("dfl", [P, TC], _F32, kind="ExternalInput").ap()
    xt_d = nc.dram_tensor("xt", [P, PAD_NODES], _F32, kind="ExternalInput").ap()
    wt_d = nc.dram_tensor("wt", [P, D_OUT], _BF16, kind="ExternalInput").ap()
    iota_d = nc.dram_tensor("iota", [P, DPAIR], _BF16, kind="ExternalInput").ap()
    bcol_d = nc.dram_tensor("bcol", [P, D_OUT // P], _F32, kind="ExternalInput").ap()
    out_d = nc.dram_tensor("out", [D_OUT, PAD_NODES], _BF16, kind="ExternalOutput").ap()

    with tile.TileContext(nc) as tc:
        with (
            tc.tile_pool(name="const", bufs=1) as cpool,
            tc.tile_pool(name="gather", bufs=3) as gpool,
            tc.tile_pool(name="xtw", bufs=2) as xtpool,
            tc.tile_pool(name="oh", bufs=12) as ohpool,
            tc.tile_pool(name="ht", bufs=6) as htpool,
            tc.tile_pool(name="ot", bufs=4) as otpool,
            tc.tile_pool(name="psA", bufs=6, space="PSUM") as psA,
            tc.tile_pool(name="psB", bufs=2, space="PSUM") as psB,
        ):
            wt_t = cpool.tile([P, D_OUT], _BF16)
            nc.sync.dma_start(out=wt_t[:], in_=wt_d)
            bcol_t = cpool.tile([P, D_OUT // P], _F32)
            nc.sync.dma_start(out=bcol_t[:], in_=bcol_d)
            iota_t = cpool.tile([P, DPAIR], _BF16)
            nc.sync.dma_start(out=iota_t[:], in_=iota_d)
            wave_q = {}
            for g in gathers:
                w = g["w"]
                q0, q1 = wave_q.get(w, (g["qoff"], g["qoff"]))
                wave_q[w] = (min(q0, g["qoff"]), max(q1, g["qoff"] + g["nch"] * 8))
            idx_ts = {}
            for w, (q0, q1) in sorted(wave_q.items()):
                t = cpool.tile([P, q1 - q0], _I16, tag=f"idx{w}")
                nc.sync.dma_start(out=t[:], in_=idx_d[:, q0:q1])
                idx_ts[w] = (t, q0)
            dfl_t = cpool.tile([P, TC], _F32)
            nc.sync.dma_start(out=dfl_t[:], in_=dfl_d)
            ndfl_t = cpool.tile([P, TC], _F32)
            nc.vector.tensor_scalar(
                out=ndfl_t[:], in0=dfl_t[:], scalar1=-1.0, scalar2=None,
                op0=mybir.AluOpType.mult,
            )

            for w, wp in enumerate(waves):
                gts = {}
                wgs = [g for g in gathers if g["w"] == w]
                for g in sorted(wgs, key=lambda g: (g["b"] == 0)):
                    b = g["b"]
                    nch = g["nch"]
                    b0 = b * SRC_BUCKET
                    b1 = min(b0 + SRC_BUCKET, N_NODES)
                    gt = gpool.tile([P, nch * P], _BF16, tag=f"g{b}")
                    nc.gpsimd.dma_gather(
                        gt[:].rearrange("p (c e) -> p c e", e=P),
                        x_d[b0:b1, :],
                        idx_ts[w][0][:, g["qoff"] - idx_ts[w][1] : g["qoff"] - idx_ts[w][1] + nch * 8],
                        nch * P,
                        nch * P,
                        P,
                        single_packet=False,
                        queue_num=b,
                    )
                    gts[b] = gt
                xtw = xtpool.tile([P, len(wp) * DPAIR], _F32)
                nc.sync.dma_start(
                    out=xtw[:],
                    in_=xt_d[:, wp[0] * DPAIR : (wp[-1] + 1) * DPAIR],
                )
                assert len(wp) % 2 == 0
                for pi, p in enumerate(wp):
                    if pi % 2 == 0:
                        ht = htpool.tile([P, 2 * DPAIR], _BF16)
                    chunks = pair_chunks[p]
                    psT = psA.tile([P, DPAIR], _F32)
                    for k, (gi, lc, ci) in enumerate(chunks):
                        oh = ohpool.tile([P, DPAIR], _BF16)
                        if k % 5 == 4:
                            tmp = ohpool.tile([P, DPAIR], _BF16, tag="ohtmp")
                            nc.scalar.activation(
                                out=tmp[:],
                                in_=iota_t[:],
                                func=mybir.ActivationFunctionType.Abs,
                                bias=ndfl_t[:, ci : ci + 1],
                                scale=1.0,
                            )
                            nc.scalar.activation(
                                out=oh[:],
                                in_=tmp[:],
                                func=mybir.ActivationFunctionType.Relu,
                                bias=1.0,
                                scale=-1.0,
                            )
                        else:
                            nc.vector.tensor_scalar(
                                out=oh[:],
                                in0=iota_t[:],
                                scalar1=dfl_t[:, ci : ci + 1],
                                scalar2=None,
                                op0=mybir.AluOpType.is_equal,
                            )
                        nc.tensor.matmul(
                            out=psT[:],
                            lhsT=gts[gathers[gi]["b"]][:, lc * P : (lc + 1) * P],
                            rhs=oh[:],
                            start=(k == 0),
                            stop=(k == len(chunks) - 1),
                        )
                    nc.vector.tensor_tensor(
                        out=ht[:, (pi % 2) * DPAIR : (pi % 2 + 1) * DPAIR],
                        in0=psT[:],
                        in1=xtw[:, pi * DPAIR : (pi + 1) * DPAIR],
                        op=mybir.AluOpType.add,
                    )
                    if pi % 2 == 0:
                        continue
                    ot = otpool.tile([P, (D_OUT // P) * 2 * DPAIR], _BF16)
                    for h in range(D_OUT // P):
                        psO = psB.tile([P, 2 * DPAIR], _F32)
                        nc.tensor.matmul(
                            out=psO[:],
                            lhsT=wt_t[:, h * P : (h + 1) * P],
                            rhs=ht[:],
                            start=True,
                            stop=True,
                        )
                        nc.scalar.activation(
                            out=ot[:, h * 2 * DPAIR : (h + 1) * 2 * DPAIR],
                            in_=psO[:],
                            func=mybir.ActivationFunctionType.Identity,
                            bias=bcol_t[:, h : h + 1],
                            scale=1.0,
                        )
                    nc.sync.dma_start(
                        out=out_d[:, (p - 1) * DPAIR : (p + 1) * DPAIR].rearrange(
                            "(h q) n -> q h n", q=P
                        ),
                        in_=ot[:].rearrange("q (h n) -> q h n", h=D_OUT // P),
                    )
    nc.compile()
    return nc


def _run(inputs, trace=False):
    import ml_dtypes
    x = np.ascontiguousarray(np.asarray(inputs["x"], dtype=np.float32))
    xb = np.ascontiguousarray(x.astype(ml_dtypes.bfloat16))
    W = np.asarray(inputs["W"], dtype=np.float32)
    b = np.asarray(inputs["b"], dtype=np.float32)

    idx_all, dfl_all, gathers, pair_chunks, waves, TC, qcols, perm = _host_prep(
        inputs["edge_index"]
    )

    WT = np.ascontiguousarray(W.T).astype(ml_dtypes.bfloat16)
    bcol = np.ascontiguousarray(b.reshape(D_OUT // P, P).T).astype(np.float32)
    iota = np.ascontiguousarray(
        np.broadcast_to(np.arange(DPAIR, dtype=np.float32)[None, :], (P, DPAIR))
    ).astype(ml_dtypes.bfloat16)

    in_maps = []
    for c in range(N_CORES):
        xt = np.zeros((P, PAD_NODES), dtype=np.float32)
        pc = perm[c]
        v = pc >= 0
        xt[:, v] = x[pc[v]].T
        in_maps.append(
            {
                "x": xb,
                "idx": np.ascontiguousarray(idx_all[c]),
                "dfl": np.ascontiguousarray(dfl_all[c]),
                "xt": xt,
                "wt": WT,
                "bcol": bcol,
                "iota": iota,
            }
        )

    nc = _build_program(gathers, pair_chunks, waves, TC, qcols)
    res = bass_utils.run_bass_kernel_spmd(
        nc, in_maps, core_ids=list(range(N_CORES)), trace=trace
    )
    out = np.empty((N_NODES, D_OUT), dtype=np.float32)
    for c in range(N_CORES):
        pc = perm[c]
        v = pc >= 0
        out[pc[v]] = res.results[c]["out"][:, v].T.astype(np.float32)
    return out, res


def kernel(**inputs):
    out, _ = _run(inputs, trace=False)
    return out


# revision 35
# speedup vs baseline: 1.1807x; 1.0107x over previous
"""GPNConv (GNN message passing) Trainium2 Bass kernel.

  agg = segment_sum(x[col], row, N)        # [N, 128]
  out = (x + agg) @ W.T + b                # [N, 512]

Sharding: destination nodes split across 8 cores (12500 each); no
cross-core communication. Per core, edges are grouped by 256-node
destination window ("pair") and by 25000-row source bucket (dma_gather
indices are int16). Each core bulk-gathers its neighbor rows x[col]
from a replicated x via dma_gather, segment-sums them with one-hot
matmuls on the PE (one-hot built on-chip from dest slots via is_equal
against a host-supplied iota row), adds the residual from a
host-transposed x-shard, applies the 128->512 linear + bias, and
writes its output shard in bf16.

Destinations are bin-packed on the host: each core's 12500 dest nodes
are assigned to its 49 pairs so that every (pair, bucket) edge count
stays at/below a multiple of 128, which keeps the uniform cross-core
chunk schedule near the 75000/128 minimum. The dest permutation is
undone on the host after the run. Gather indices are sorted within
each group for HBM locality.
"""

import hashlib
import os

import numpy as np

import concourse.bass as bass
import concourse.mybir as mybir
import concourse.tile as tile
from concourse import bacc
from concourse import bass_utils

P = 128
N_NODES = 100000
D_IN = 128
D_OUT = 512
N_CORES = 8
NODES_PER_CORE = N_NODES // N_CORES             # 12500
DPAIR = 256                                      # dest window (psum free dim)
PAIRS_PER_CORE = 50                              # 12800 slots for 12500 dests
PAD_NODES = PAIRS_PER_CORE * DPAIR               # 12800
WAVE_SIZES = [4, 8, 12, 12, 10, 4]               # pairs per gather wave
N_BUCKETS = 4
SRC_BUCKET = 25000                               # rows per source bucket (int16 range)
CAP = 384                                        # max edges per (pair, bucket): 3 chunks
PAD_SLOT = 999.0                                 # one-hot slot matching nothing
SCHED_CACHE = "/tmp/gpn_sched_v3.npz"

_F32 = mybir.dt.float32
_BF16 = mybir.dt.bfloat16
_I16 = mybir.dt.int16


def _assign_dests(row, col):
    """Assign dest nodes to (core, pair, slot) so that per-(core,pair,bucket)
    edge counts pack tightly into 128-edge chunks.

    Returns perm[core, PAD_NODES] = global dest id per slot (-1 for empty)
    and counts[core, PAIRS, NB]."""
    bucket = col // SRC_BUCKET
    # per-dest degree vector over buckets
    dv = np.zeros((N_NODES, N_BUCKETS), dtype=np.int32)
    np.add.at(dv, (row, bucket), 1)
    deg = dv.sum(axis=1)

    # dest -> core: snake-deal in degree-desc order balances per-core bucket sums
    order = np.argsort(-deg, kind="stable")
    core_of = np.empty(N_NODES, dtype=np.int32)
    pos = np.arange(N_NODES)
    rnd = pos // N_CORES
    lane = pos % N_CORES
    lane = np.where(rnd % 2 == 1, N_CORES - 1 - lane, lane)
    core_of[order] = lane

    perm = np.full((N_CORES, PAD_NODES), -1, dtype=np.int64)
    counts = np.zeros((N_CORES, PAIRS_PER_CORE, N_BUCKETS), dtype=np.int32)
    for c in range(N_CORES):
        dests = np.where(core_of == c)[0]
        dvc = dv[dests]
        dorder = np.argsort(-dvc.sum(axis=1), kind="stable")
        cnt = np.zeros((PAIRS_PER_CORE, N_BUCKETS), dtype=np.int64)
        slots = np.zeros(PAIRS_PER_CORE, dtype=np.int64)
        assign = np.empty(len(dests), dtype=np.int64)
        for i in dorder:
            v = dvc[i]
            new = cnt + v[None, :]
            # balance buckets within the hard CAP; prefer the fullest
            # feasible pair so later (big) dests keep room
            score = (new / CAP).max(axis=1)
            score[(new > CAP).any(axis=1)] = 1e18
            score[slots >= DPAIR] = 1e18
            p = int(np.argmin(score))
            if score[p] >= 1e18:  # no feasible pair: least-overflow fallback
                over = np.maximum(new - CAP, 0).sum(axis=1).astype(np.float64)
                over[slots >= DPAIR] = 1e18
                p = int(np.argmin(over))
            assign[i] = p
            cnt[p] += v
            slots[p] += 1
        # slot positions within each pair, in assignment order
        for p in range(PAIRS_PER_CORE):
            mask = assign == p
            ds = dests[mask]
            perm[c, p * DPAIR : p * DPAIR + len(ds)] = ds
        counts[c] = cnt
    return perm, counts


def _host_prep(edge_index):
    """Group edges by (core, pair, bucket) under a bin-packed dest layout;
    build uniform chunk schedule, int16 gather-index array (sorted within
    groups) and bf16 dest-slot array per core."""
    row = np.asarray(edge_index[0], dtype=np.int64)
    col = np.asarray(edge_index[1], dtype=np.int64)

    key_h = hashlib.sha1(np.ascontiguousarray(edge_index)).hexdigest()
    perm = None
    if os.path.exists(SCHED_CACHE):
        try:
            z = np.load(SCHED_CACHE, allow_pickle=False)
            if str(z["key"]) == key_h:
                perm, counts = z["perm"], z["counts"]
        except Exception:
            perm = None
    if perm is None:
        perm, counts = _assign_dests(row, col)
        try:
            np.savez(SCHED_CACHE, key=key_h, perm=perm, counts=counts)
        except Exception:
            pass

    # slot_of[global dest] = core * PAD_NODES + slot
    slot_of = np.empty(N_NODES, dtype=np.int64)
    valid = perm >= 0
    flat_idx = np.arange(N_CORES * PAD_NODES).reshape(N_CORES, PAD_NODES)
    slot_of[perm[valid]] = flat_idx[valid]

    s = slot_of[row]
    core = s // PAD_NODES
    local = s % PAD_NODES
    pair = local // DPAIR                         # 0..48
    pslot = local % DPAIR                         # 0..255
    bucket = col // SRC_BUCKET
    brel = (col % SRC_BUCKET).astype(np.int16)

    key = (core * PAIRS_PER_CORE + pair) * N_BUCKETS + bucket
    ngroups = N_CORES * PAIRS_PER_CORE * N_BUCKETS
    budget = -(-counts.max(axis=0) // P)          # [PAIRS, NB] ceil

    # sort edges by (group, brel) for HBM locality within each chunk run
    order = np.lexsort((brel, key))
    brel_s = brel[order]
    pslot_s = pslot[order]
    key_s = key[order]
    starts = np.searchsorted(key_s, np.arange(ngroups + 1))

    assert sum(WAVE_SIZES) == PAIRS_PER_CORE
    waves = []
    p0 = 0
    for ws in WAVE_SIZES:
        waves.append(list(range(p0, p0 + ws)))
        p0 += ws
    gathers = []      # per (w,b): dict(nch, qoff, coff, bucket, wave)
    pair_chunks = {}  # pair -> list of (gather idx, local_chunk, ci)
    ci = 0
    qcols = 0
    for w, wp in enumerate(waves):
        for b in range(N_BUCKETS):
            nch = int(sum(budget[p][b] for p in wp))
            if nch == 0:
                continue
            g = dict(w=w, b=b, nch=nch, qoff=qcols, coff=ci)
            gathers.append(g)
            lc = 0
            for p in wp:
                for j in range(int(budget[p][b])):
                    pair_chunks.setdefault(p, []).append((len(gathers) - 1, lc, ci))
                    lc += 1
                    ci += 1
            qcols += nch * 8  # (nch*128 idxs)/16
    TC = ci

    idx_all = np.zeros((N_CORES, 16, qcols), dtype=np.int16)
    dfl_all = np.full((N_CORES, P, TC), PAD_SLOT, dtype=np.float32)
    for c in range(N_CORES):
        for g in gathers:
            w, b = g["w"], g["b"]
            lc = 0
            for p in waves[w]:
                gk = (c * PAIRS_PER_CORE + p) * N_BUCKETS + b
                b0, b1 = starts[gk], starts[gk + 1]
                n = b1 - b0
                kb = int(budget[p][b])
                assert n <= kb * P, (c, p, b, n, kb)
                if n:
                    i = (lc + np.arange(n) // P) * P + np.arange(n) % P
                    idx_all[c, i % 16, g["qoff"] + i // 16] = brel_s[b0:b1]
                    dfl_all[c, np.arange(n) % P, g["coff"] + lc + np.arange(n) // P] = (
                        pslot_s[b0:b1]
                    )
                lc += kb
    # replicate idx rows to 128 partitions (8 Q7 cores x 16-partition stripes)
    idx_all = np.tile(idx_all, (1, 8, 1))
    return idx_all, dfl_all, gathers, pair_chunks, waves, TC, qcols, perm


def _build_program(gathers, pair_chunks, waves, TC, qcols):
    nc = bacc.Bacc(
        "TRN2",
        target_bir_lowering=False,
        debug=False,
        enable_asserts=False,
        num_devices=N_CORES,
        num_swdge_queues=4,
        dynamic_dma_scratch_size=24576,
    )
    x_d = nc.dram_tensor("x", [N_NODES, D_IN], _BF16, kind="ExternalInput").ap()
    idx_d = nc.dram_tensor("idx", [P, qcols], _I16, kind="ExternalInput").ap()
    dfl_d = nc.dram_tensor("dfl", [P, TC], _F32, kind="ExternalInput").ap()
    xt_d = nc.dram_tensor("xt", [P, PAD_NODES], _F32, kind="ExternalInput").ap()
    wt_d = nc.dram_tensor("wt", [P, D_OUT], _BF16, kind="ExternalInput").ap()
    iota_d = nc.dram_tensor("iota", [P, DPAIR], _BF16, kind="ExternalInput").ap()
    bcol_d = nc.dram_tensor("bcol", [P, D_OUT // P], _F32, kind="ExternalInput").ap()
    out_d = nc.dram_tensor("out", [D_OUT, PAD_NODES], _BF16, kind="ExternalOutput").ap()

    with tile.TileContext(nc) as tc:
        with (
            tc.tile_pool(name="const", bufs=1) as cpool,
            tc.tile_pool(name="gather", bufs=3) as gpool,
            tc.tile_pool(name="xtw", bufs=2) as xtpool,
            tc.tile_pool(name="oh", bufs=12) as ohpool,
            tc.tile_pool(name="ht", bufs=6) as htpool,
            tc.tile_pool(name="ot", bufs=4) as otpool,
            tc.tile_pool(name="psA", bufs=6, space="PSUM") as psA,
            tc.tile_pool(name="psB", bufs=2, space="PSUM") as psB,
        ):
            wt_t = cpool.tile([P, D_OUT], _BF16)
            nc.sync.dma_start(out=wt_t[:], in_=wt_d)
            bcol_t = cpool.tile([P, D_OUT // P], _F32)
            nc.sync.dma_start(out=bcol_t[:], in_=bcol_d)
            iota_t = cpool.tile([P, DPAIR], _BF16)
            nc.sync.dma_start(out=iota_t[:], in_=iota_d)
            warm_idx = cpool.tile([P, 8], _I16)
            nc.vector.memset(warm_idx[:], 0)
            warm_gt = cpool.tile([P, P], _BF16)
            nc.gpsimd.dma_gather(
                warm_gt[:].rearrange("p (c e) -> p c e", e=P),
                x_d[0:SRC_BUCKET, :],
                warm_idx[:],
                P,
                P,
                P,
                single_packet=False,
                queue_num=0,
            )
            wave_q = {}
            for g in gathers:
                w = g["w"]
                q0, q1 = wave_q.get(w, (g["qoff"], g["qoff"]))
                wave_q[w] = (min(q0, g["qoff"]), max(q1, g["qoff"] + g["nch"] * 8))
            idx_ts = {}
            for w, (q0, q1) in sorted(wave_q.items()):
                t = cpool.tile([P, q1 - q0], _I16, tag=f"idx{w}")
                nc.sync.dma_start(out=t[:], in_=idx_d[:, q0:q1])
                idx_ts[w] = (t, q0)
            dfl_t = cpool.tile([P, TC], _F32)
            nc.sync.dma_start(out=dfl_t[:], in_=dfl_d)
            ndfl_t = cpool.tile([P, TC], _F32)
            nc.vector.tensor_scalar(
                out=ndfl_t[:], in0=dfl_t[:], scalar1=-1.0, scalar2=None,
                op0=mybir.AluOpType.mult,
            )

            for w, wp in enumerate(waves):
                gts = {}
                wgs = [g for g in gathers if g["w"] == w]
                for g in sorted(wgs, key=lambda g: (g["b"] == 0)):
                    b = g["b"]
                    nch = g["nch"]
                    b0 = b * SRC_BUCKET
                    b1 = min(b0 + SRC_BUCKET, N_NODES)
                    gt = gpool.tile([P, nch * P], _BF16, tag=f"g{b}")
                    nc.gpsimd.dma_gather(
                        gt[:].rearrange("p (c e) -> p c e", e=P),
                        x_d[b0:b1, :],
                        idx_ts[w][0][:, g["qoff"] - idx_ts[w][1] : g["qoff"] - idx_ts[w][1] + nch * 8],
                        nch * P,
                        nch * P,
                        P,
                        single_packet=False,
                        queue_num=b,
                    )
                    gts[b] = gt
                xtw = xtpool.tile([P, len(wp) * DPAIR], _F32)
                nc.sync.dma_start(
                    out=xtw[:],
                    in_=xt_d[:, wp[0] * DPAIR : (wp[-1] + 1) * DPAIR],
                )
                assert len(wp) % 2 == 0
                for pi, p in enumerate(wp):
                    if pi % 2 == 0:
                        ht = htpool.tile([P, 2 * DPAIR], _BF16)
                    chunks = pair_chunks[p]
                    psT = psA.tile([P, DPAIR], _F32)
                    for k, (gi, lc, ci) in enumerate(chunks):
                        oh = ohpool.tile([P, DPAIR], _BF16)
                        if k % 5 == 4:
                            tmp = ohpool.tile([P, DPAIR], _BF16, tag="ohtmp")
                            nc.scalar.activation(
                                out=tmp[:],
                                in_=iota_t[:],
                                func=mybir.ActivationFunctionType.Abs,
                                bias=ndfl_t[:, ci : ci + 1],
                                scale=1.0,
                            )
                            nc.scalar.activation(
                                out=oh[:],
                                in_=tmp[:],
                                func=mybir.ActivationFunctionType.Relu,
                                bias=1.0,
                                scale=-1.0,
                            )
                        else:
                            nc.vector.tensor_scalar(
                                out=oh[:],
                                in0=iota_t[:],
                                scalar1=dfl_t[:, ci : ci + 1],
                                scalar2=None,
                                op0=mybir.AluOpType.is_equal,
                            )
                        nc.tensor.matmul(
                            out=psT[:],
                            lhsT=gts[gathers[gi]["b"]][:, lc * P : (lc + 1) * P],
                            rhs=oh[:],
                            start=(k == 0),
                            stop=(k == len(chunks) - 1),
                        )
                    nc.vector.tensor_tensor(
                        out=ht[:, (pi % 2) * DPAIR : (pi % 2 + 1) * DPAIR],
                        in0=psT[:],
                        in1=xtw[:, pi * DPAIR : (pi + 1) * DPAIR],
                        op=mybir.AluOpType.add,
                    )
                    if pi % 2 == 0:
                        continue
                    ot = otpool.tile([P, (D_OUT // P) * 2 * DPAIR], _BF16)
                    for h in range(D_OUT // P):
                        psO = psB.tile([P, 2 * DPAIR], _F32)
                        nc.tensor.matmul(
                            out=psO[:],
                            lhsT=wt_t[:, h * P : (h + 1) * P],
                            rhs=ht[:],
                            start=True,
                            stop=True,
                        )
                        nc.scalar.activation(
                            out=ot[:, h * 2 * DPAIR : (h + 1) * 2 * DPAIR],
                            in_=psO[:],
                            func=mybir.ActivationFunctionType.Identity,
                            bias=bcol_t[:, h : h + 1],
                            scale=1.0,
                        )
                    nc.sync.dma_start(
                        out=out_d[:, (p - 1) * DPAIR : (p + 1) * DPAIR].rearrange(
                            "(h q) n -> q h n", q=P
                        ),
                        in_=ot[:].rearrange("q (h n) -> q h n", h=D_OUT // P),
                    )
    nc.compile()
    return nc


def _run(inputs, trace=False):
    import ml_dtypes
    x = np.ascontiguousarray(np.asarray(inputs["x"], dtype=np.float32))
    xb = np.ascontiguousarray(x.astype(ml_dtypes.bfloat16))
    W = np.asarray(inputs["W"], dtype=np.float32)
    b = np.asarray(inputs["b"], dtype=np.float32)

    idx_all, dfl_all, gathers, pair_chunks, waves, TC, qcols, perm = _host_prep(
        inputs["edge_index"]
    )

    WT = np.ascontiguousarray(W.T).astype(ml_dtypes.bfloat16)
    bcol = np.ascontiguousarray(b.reshape(D_OUT // P, P).T).astype(np.float32)
    iota = np.ascontiguousarray(
        np.broadcast_to(np.arange(DPAIR, dtype=np.float32)[None, :], (P, DPAIR))
    ).astype(ml_dtypes.bfloat16)

    in_maps = []
    for c in range(N_CORES):
        xt = np.zeros((P, PAD_NODES), dtype=np.float32)
        pc = perm[c]
        v = pc >= 0
        xt[:, v] = x[pc[v]].T
        in_maps.append(
            {
                "x": xb,
                "idx": np.ascontiguousarray(idx_all[c]),
                "dfl": np.ascontiguousarray(dfl_all[c]),
                "xt": xt,
                "wt": WT,
                "bcol": bcol,
                "iota": iota,
            }
        )

    nc = _build_program(gathers, pair_chunks, waves, TC, qcols)
    res = bass_utils.run_bass_kernel_spmd(
        nc, in_maps, core_ids=list(range(N_CORES)), trace=trace
    )
    out = np.empty((N_NODES, D_OUT), dtype=np.float32)
    for c in range(N_CORES):
        pc = perm[c]
        v = pc >= 0
        out[pc[v]] = res.results[c]["out"][:, v].T.astype(np.float32)
    return out, res


def kernel(**inputs):
    out, _ = _run(inputs, trace=False)
    return out
